# revision 3
# baseline (speedup 1.0000x reference)
"""Cross-attention kernel for Trainium2, SPMD across 8 NeuronCores.

Problem: B=4, N=M=2048, QD=1024, CD=768, H=8, DH=64, INNER=512 (f32).
  q = x @ Wq; k = ctx @ Wk; v = ctx @ Wv
  out = softmax(q k^T / sqrt(DH)) v @ Wo + bo

Sharding: batch x query-halves -> 8 shards. Core c handles batch c//2,
query rows (c%2)*1024:(c%2+1)*1024, with that batch's full context.
Each core computes a disjoint (1024, 1024) slice of the output; no
cross-core communication. Weights are replicated.

Per-core layout (everything fed transposed so the contraction dim lands
on SBUF partitions; no on-device transposes anywhere):
  qT = scale * Wq^T @ xT          [INNER, n]   (bf16)
  kT = Wk^T @ ctxT                [INNER, m]   (bf16)
  v  = ctxT^T @ Wv (+ones col)    [m, 8*65]    (bf16)
  sT_h = kT_h^T q: one K=64 matmul per (m-tile, n-block); the two heads
     of an INNER partition-tile are issued adjacently so the PE runs
     them concurrently in 64x128 row-tiling mode.
  E = exp(sT) on ScalarE (no max subtraction: |s| < 3 for this data)
  O'_h = v_aug^T @ E  -> [65, n] psum; row 64 = softmax denominators r
  O_h = O'_h * (1/r broadcast)    [65, n] bf16 (row 64 -> 1, harmless)
  out = sum_h O_h^T @ wo_h + bo   (wo_h row 64 zeroed; bo via K=1 matmul)
"""

import numpy as np

B, N, M = 4, 2048, 2048
QD, CD = 1024, 768
H, DH = 8, 64
INNER = H * DH  # 512
NS = 1024  # query rows per core
SCALE = DH ** -0.5

_CACHED_NC = None


def build_nc():
    import concourse.bacc as bacc
    import concourse.mybir as mybir
    import concourse.tile as tile

    f32 = mybir.dt.float32
    bf16 = mybir.dt.bfloat16
    FT = mybir.ActivationFunctionType
    AluOp = mybir.AluOpType

    nc = bacc.Bacc(None)
    xT_d = nc.dram_tensor("xT", (QD, NS), f32, kind="ExternalInput")
    ctxT_d = nc.dram_tensor("ctxT", (CD, M), f32, kind="ExternalInput")
    Wq_d = nc.dram_tensor("Wq", (QD, INNER), f32, kind="ExternalInput")
    Wk_d = nc.dram_tensor("Wk", (CD, INNER), f32, kind="ExternalInput")
    Wv_d = nc.dram_tensor("Wv", (CD, INNER), f32, kind="ExternalInput")
    Wo_d = nc.dram_tensor("Wo", (INNER, QD), f32, kind="ExternalInput")
    bo_d = nc.dram_tensor("bo", (1, QD), f32, kind="ExternalInput")
    out_d = nc.dram_tensor("out", (NS, QD), f32, kind="ExternalOutput")

    KQ = QD // 128   # 8 k-tiles for q projection
    KC = CD // 128   # 6 k-tiles for k/v projections
    NI = INNER // 128  # 4 partition tiles of INNER
    MT = M // 128    # 16 context tiles
    NB = NS // 512   # 2 query blocks
    VW = 65          # v columns per head incl. ones column

    with tile.TileContext(nc) as tc:
        with (
            tc.tile_pool(name="w", bufs=1) as wp,
            tc.tile_pool(name="a", bufs=1) as ap,
            tc.tile_pool(name="e", bufs=40) as ep,
            tc.tile_pool(name="s", bufs=2) as sp,
            tc.tile_pool(name="o", bufs=2) as op_,
            tc.tile_pool(name="ps", bufs=2, space="PSUM") as pp2,
            tc.tile_pool(name="pss", bufs=4, space="PSUM") as pp4,
        ):
            # ---- weight / input loads (SWDGE casts f32 -> bf16) ----
            wq = [wp.tile([128, INNER], bf16, tag=f"wq{k}", name=f"wq{k}") for k in range(KQ)]
            for k in range(KQ):
                nc.gpsimd.dma_start(wq[k][:], Wq_d[k * 128:(k + 1) * 128, :])
            xT = [wp.tile([128, NS], bf16, tag=f"xT{k}", name=f"xT{k}") for k in range(KQ)]
            for k in range(KQ):
                nc.gpsimd.dma_start(xT[k][:], xT_d[k * 128:(k + 1) * 128, :])
            wk = [wp.tile([128, INNER], bf16, tag=f"wk{k}", name=f"wk{k}") for k in range(KC)]
            wv = [wp.tile([128, INNER], bf16, tag=f"wv{k}", name=f"wv{k}") for k in range(KC)]
            for k in range(KC):
                nc.gpsimd.dma_start(wk[k][:], Wk_d[k * 128:(k + 1) * 128, :])
                nc.gpsimd.dma_start(wv[k][:], Wv_d[k * 128:(k + 1) * 128, :])
            ctxT = [wp.tile([128, M], bf16, tag=f"cT{k}", name=f"cT{k}") for k in range(KC)]
            for k in range(KC):
                nc.gpsimd.dma_start(ctxT[k][:], ctxT_d[k * 128:(k + 1) * 128, :])
            # per-head Wo with a zeroed 65th row (so O row 64 contributes 0)
            wo = [wp.tile([VW, QD], bf16, tag=f"wo{h}", name=f"wo{h}") for h in range(H)]
            for h in range(H):
                nc.gpsimd.dma_start(wo[h][0:64, :], Wo_d[h * 64:(h + 1) * 64, :])
                nc.vector.memset(wo[h][64:65, :], 0.0)
            bo_sb = wp.tile([1, QD], bf16, tag="bo", name="bo_sb")
            nc.gpsimd.dma_start(bo_sb[:], bo_d[:])
            ones_col = wp.tile([1, 128], bf16, tag="ones", name="ones_col")
            nc.vector.memset(ones_col[:], 1.0)

            # ---- qT = SCALE * Wq^T @ xT : [INNER, NS] bf16 ----
            qT = [ap.tile([128, NS], bf16, tag=f"qT{j}", name=f"qT{j}") for j in range(NI)]
            for j in range(NI):
                for nb in range(NB):
                    ps = pp2.tile([128, 512], f32, tag="pp", name="pp")
                    for k in range(KQ):
                        nc.tensor.matmul(
                            ps[:], wq[k][:, j * 128:(j + 1) * 128],
                            xT[k][:, nb * 512:(nb + 1) * 512],
                            start=(k == 0), stop=(k == KQ - 1))
                    nc.scalar.activation(qT[j][:, nb * 512:(nb + 1) * 512],
                                         ps[:], FT.Copy, scale=SCALE)

            # ---- kT = Wk^T @ ctxT : [INNER, M] bf16 ----
            kT = [ap.tile([128, M], bf16, tag=f"kT{j}", name=f"kT{j}") for j in range(NI)]
            for j in range(NI):
                for mb in range(M // 512):
                    ps = pp2.tile([128, 512], f32, tag="pp", name="pp")
                    for k in range(KC):
                        nc.tensor.matmul(
                            ps[:], wk[k][:, j * 128:(j + 1) * 128],
                            ctxT[k][:, mb * 512:(mb + 1) * 512],
                            start=(k == 0), stop=(k == KC - 1))
                    nc.vector.tensor_copy(kT[j][:, mb * 512:(mb + 1) * 512], ps[:])

            # ---- v = ctxT^T @ Wv : [M, 8*65] bf16, ones col per head ----
            v = [ap.tile([128, H * VW], bf16, tag=f"v{t}", name=f"v{t}") for t in range(MT)]
            for t in range(MT):
                ps = pp2.tile([128, 512], f32, tag="pp", name="pp")
                for k in range(KC):
                    nc.tensor.matmul(
                        ps[:], ctxT[k][:, t * 128:(t + 1) * 128], wv[k][:],
                        start=(k == 0), stop=(k == KC - 1))
                v3 = v[t][:].rearrange("p (h d) -> p h d", d=VW)
                nc.vector.tensor_copy(
                    v3[:, :, 0:64], ps[:].rearrange("p (h d) -> p h d", d=64))
                nc.vector.memset(v3[:, :, 64:65], 1.0)

            # ---- attention, head pair j = heads (2j, 2j+1) ----
            On = [ap.tile([VW, NS], bf16, tag=f"On{h}", name=f"On{h}") for h in range(H)]
            for j in range(NI):
                for nb in range(NB):
                    E = {}
                    for t in range(MT):
                        for hh in range(2):
                            ps = pp4.tile([128, 512], f32, tag="pss", name="pss")
                            nc.tensor.matmul(
                                ps[:],
                                kT[j][hh * 64:(hh + 1) * 64, t * 128:(t + 1) * 128],
                                qT[j][hh * 64:(hh + 1) * 64, nb * 512:(nb + 1) * 512],
                                start=True, stop=True)
                            e = ep.tile([128, 512], bf16, tag="E", name="E")
                            nc.scalar.activation(e[:], ps[:], FT.Exp)
                            E[(hh, t)] = e
                    for hh in range(2):
                        h = 2 * j + hh
                        po = pp2.tile([VW, 512], f32, tag="po", name="po")
                        for t in range(MT):
                            nc.tensor.matmul(
                                po[:], v[t][:, h * VW:(h + 1) * VW], E[(hh, t)][:],
                                start=(t == 0), stop=(t == MT - 1))
                        rr = sp.tile([1, 512], f32, tag="rr", name="rr")
                        nc.vector.reciprocal(rr[:], po[64:65, :])
                        rb = sp.tile([VW, 512], f32, tag="rb", name="rb")
                        nc.gpsimd.partition_broadcast(rb[:], rr[:], channels=VW)
                        nc.vector.tensor_tensor(
                            On[h][:, nb * 512:(nb + 1) * 512], po[:], rb[:],
                            op=AluOp.mult)

            # ---- out = sum_h O_h^T @ wo_h + bo ----
            for nt in range(NS // 128):
                ot = op_.tile([128, QD], f32, tag="ot", name="ot")
                for qb in range(QD // 512):
                    pf = pp2.tile([128, 512], f32, tag="pp", name="pf")
                    for h in range(H):
                        nc.tensor.matmul(
                            pf[:], On[h][:, nt * 128:(nt + 1) * 128],
                            wo[h][:, qb * 512:(qb + 1) * 512],
                            start=(h == 0), stop=False)
                    nc.tensor.matmul(
                        pf[:], ones_col[:], bo_sb[:, qb * 512:(qb + 1) * 512],
                        start=False, stop=True)
                    nc.vector.tensor_copy(ot[:, qb * 512:(qb + 1) * 512], pf[:])
                nc.sync.dma_start(out_d[nt * 128:(nt + 1) * 128, :], ot[:])

    nc.compile()
    return nc


def _get_nc():
    global _CACHED_NC
    if _CACHED_NC is None:
        _CACHED_NC = build_nc()
    return _CACHED_NC


def _shard_inputs(x, context, Wq, Wk, Wv, Wo, bo):
    f = np.float32
    Wq = np.ascontiguousarray(Wq, dtype=f)
    Wk = np.ascontiguousarray(Wk, dtype=f)
    Wv = np.ascontiguousarray(Wv, dtype=f)
    Wo = np.ascontiguousarray(Wo, dtype=f)
    bo2 = np.ascontiguousarray(np.asarray(bo, dtype=f).reshape(1, QD))
    in_maps = []
    for c in range(8):
        b, q = divmod(c, 2)
        in_maps.append({
            "xT": np.ascontiguousarray(
                np.asarray(x[b, q * NS:(q + 1) * NS, :], dtype=f).T),
            "ctxT": np.ascontiguousarray(np.asarray(context[b], dtype=f).T),
            "Wq": Wq, "Wk": Wk, "Wv": Wv, "Wo": Wo, "bo": bo2,
        })
    return in_maps


def kernel(x, context, Wq, Wk, Wv, Wo, bo, _trace=False):
    from concourse.bass_utils import run_bass_kernel_spmd

    nc = _get_nc()
    in_maps = _shard_inputs(x, context, Wq, Wk, Wv, Wo, bo)
    res = run_bass_kernel_spmd(nc, in_maps, core_ids=list(range(8)),
                               trace=_trace)
    out = np.empty((B, N, QD), np.float32)
    for c in range(8):
        b, q = divmod(c, 2)
        out[b, q * NS:(q + 1) * NS, :] = res.results[c]["out"]
    if _trace:
        kernel._last_result = res
    return out
